# revision 1
# baseline (speedup 1.0000x reference)
"""Causal self-attention (B=2, T=2048, C=2048, H=16, rope) on 8 trn2 cores.

Sharding: tensor-parallel over heads. Each core owns 2 of 16 heads:
  - Wqkv columns for its heads (q,k,v), Wproj rows for its heads.
  - Computes qkv projection, rope, causal attention, and its partial
    output projection y_c = O_c @ Wproj_c  (full [4096, 2048]).
  - Host sums the 8 partials (the all-reduce / unshard for row-parallel TP).

All matmul operands fp16 (PE full rate; fp32 is 1/4 rate), fp32 PSUM
accumulation. Layouts keep the pipeline transpose-free except one 128x128
PE transpose per output tile (O -> O^T for the projection). Softmax sums
come free as a ones-column appended to V; normalization is applied to O
before projection. Engine balance: exp owns the scalar engine (it is the
phase-C rate limiter); all other PSUM drains go to DVE/Pool/scalar by
phase load. Rope reads the qk PSUM directly with partition-shifted DVE
muls (legal when one input is PSUM), so there is no PE rotation matmul
and no st staging copy. Cold start is hidden by issuing the wqkv/x DMAs
in fine chunks in first-use order.
"""

import sys

for _p in ("/opt/trn_rl_repo",):
    if _p not in sys.path:
        sys.path.append(_p)

import numpy as np

# ---- problem constants (hardcoded per the task contract) ----
B, T, C, H = 2, 2048, 2048, 16
D = C // H  # 128
NCORES = 8
HPC = H // NCORES  # heads per core = 2
NTOK = B * T  # 4096
P = 128
CT = C // P  # 16 contraction tiles
TOK512 = NTOK // 512  # 8
NQ = T // 512  # q-tiles per unit = 4
TT = NTOK // P  # 32 token 128-tiles
SCALE = 1.0 / np.sqrt(D)

_compiled = None

# tuning knobs (overridable before _build_bass for experiments)
KNOBS = {"cs": 2, "co": 2, "work": 2, "pt": 20, "ysb": 6, "rope": 4, "osb": 10, "xt": 2}


def _build_bass():
    import concourse.bacc as bacc
    import concourse.mybir as mybir
    import concourse.tile as tile
    from contextlib import ExitStack

    f16 = mybir.dt.float16
    f32 = mybir.dt.float32
    Exp = mybir.ActivationFunctionType.Exp

    nc = bacc.Bacc()

    xT = nc.declare_dram_parameter("xT", [C, NTOK], f16, isOutput=False)
    wqk = nc.declare_dram_parameter("wqk", [C, 2 * HPC * D], f16, isOutput=False)
    wv = nc.declare_dram_parameter("wv", [C, HPC * D], f16, isOutput=False)
    wproj = nc.declare_dram_parameter("wproj", [HPC * D, C], f16, isOutput=False)
    cos_t = nc.declare_dram_parameter("cos_t", [P, NTOK], f16, isOutput=False)
    sin_t = nc.declare_dram_parameter("sin_t", [P, NTOK], f16, isOutput=False)
    maskw = nc.declare_dram_parameter("maskw", [P, 1024], f16, isOutput=False)
    ident = nc.declare_dram_parameter("ident", [P, P], f16, isOutput=False)
    y = nc.declare_dram_parameter("y", [NTOK, C], f16, isOutput=True)

    with tile.TileContext(nc) as tc, ExitStack() as ctx:
        pers = ctx.enter_context(tc.tile_pool(name="pers", bufs=1))

        # ---- persistent SBUF tensors ----
        wqk_sb = pers.tile([P, CT, 4 * P], f16)  # [c128, ct, (q0,q1,k0,k1)*128]
        wv_sb = pers.tile([P, CT, 2 * P], f16)
        wproj_sb = pers.tile([P, HPC, C], f16)
        cos_sb = pers.tile([P, NTOK], f16)
        sin_sb = pers.tile([P, NTOK], f16)
        mask_sb = pers.tile([P, 1024], f16)
        id_sb = pers.tile([P, P], f16)
        qT_sb = pers.tile([P, HPC, NTOK], f16)  # [d, h, tok] rope'd
        kT_sb = pers.tile([P, HPC, NTOK], f16)
        v_sb = pers.tile([P, TT, HPC, D + 1], f16)  # [tokmod, tt, h, D|ones]
        oT_sb = pers.tile([P, TT, HPC, P], f16)  # [d, tt, h, tokmod]

        # ---- working pools (all open for the whole kernel: the stack
        # allocator must never reuse a released zone — released-zone deps
        # blow past the 1-wait/instruction HW limit pre-bacc-split) ----
        xt_pool = ctx.enter_context(tc.tile_pool(name="xt", bufs=KNOBS["xt"]))
        rope_pool = ctx.enter_context(tc.tile_pool(name="rope", bufs=KNOBS["rope"]))
        p_pool = ctx.enter_context(tc.tile_pool(name="pt", bufs=KNOBS["pt"]))
        osb_pool = ctx.enter_context(tc.tile_pool(name="osb", bufs=KNOBS["osb"]))
        ysb_pool = ctx.enter_context(tc.tile_pool(name="ysb", bufs=KNOBS["ysb"]))
        # PSUM (8 banks), phase-dedicated to avoid cross-phase slot stalls:
        #   work: 2-bank x2 (B: qk/v chains; D: wide yps)
        #   cs:   1-bank x2 (C: S-tiles + transposes)
        #   co:   1-bank x2 (C: per-sub O accumulation chains)
        work_pool = ctx.enter_context(tc.tile_pool(name="work", bufs=KNOBS["work"], space="PSUM"))
        cs_pool = ctx.enter_context(tc.tile_pool(name="cs", bufs=KNOBS["cs"], space="PSUM"))
        co_pool = ctx.enter_context(tc.tile_pool(name="co", bufs=KNOBS["co"], space="PSUM"))

        # DMA issue order = first-use order (one serialized HWDGE pipe):
        # wqk in fine ct-chunks, then x/rope tables for ti=0, then wv (first
        # needed by ti=0's trailing v chains), then phase-C constants.
        xts = [None] * TOK512
        xts[0] = xt_pool.tile([P, CT, 512], f16, tag="xt", name="xt")
        nc.sync.dma_start(
            wqk_sb[:, :, 0:P],
            wqk[:, 0:P].rearrange("(ct p) m -> p ct m", p=P),
        )
        for ch in range(4):
            nc.sync.dma_start(
                xts[0][:, ch * 4 : (ch + 1) * 4, :],
                xT[ch * 512 : (ch + 1) * 512, 0:512].rearrange(
                    "(ct p) j -> p ct j", p=P
                ),
            )
        for ci in range(1, 4):
            nc.sync.dma_start(
                wqk_sb[:, :, ci * P : (ci + 1) * P],
                wqk[:, ci * P : (ci + 1) * P].rearrange("(ct p) m -> p ct m", p=P),
            )
        nc.sync.dma_start(cos_sb[:, 0:512], cos_t[:, 0:512])
        nc.sync.dma_start(sin_sb[:, 0:512], sin_t[:, 0:512])
        nc.sync.dma_start(wv_sb[:], wv.rearrange("(ct p) m -> p ct m", p=P))
        nc.sync.dma_start(mask_sb[:], maskw[:])
        nc.sync.dma_start(id_sb[:], ident[:])
        nc.vector.memset(v_sb[:, :, :, D : D + 1], 1.0)

        # ======== phase B: qkv projection + rope ========
        for ti in range(TOK512):
            t0 = ti * 512
            if xts[ti] is None:
                xts[ti] = xt_pool.tile([P, CT, 512], f16, tag="xt", name="xt")
                for ch in range(4):
                    nc.sync.dma_start(
                        xts[ti][:, ch * 4 : (ch + 1) * 4, :],
                        xT[ch * 512 : (ch + 1) * 512, t0 : t0 + 512].rearrange(
                            "(ct p) j -> p ct j", p=P
                        ),
                    )
                # stream rope tables alongside
                nc.sync.dma_start(cos_sb[:, t0 : t0 + 512], cos_t[:, t0 : t0 + 512])
                nc.sync.dma_start(sin_sb[:, t0 : t0 + 512], sin_t[:, t0 : t0 + 512])
            xt = xts[ti]
            # q,k columns: out^T orientation -> [col128, tok512]
            for ci in range(4):
                hh = ci % HPC
                dstT = qT_sb if ci < HPC else kT_sb
                ps = work_pool.tile([P, 512], f32, tag="work", name="psqk")
                for ct in range(CT):
                    nc.tensor.matmul(
                        ps[:],
                        wqk_sb[:, ct, ci * P : (ci + 1) * P],
                        xt[:, ct, :],
                        start=(ct == 0),
                        stop=(ct == CT - 1),
                    )
                # rope straight off PSUM: t1 = ps*cos; t2 = roll64(ps)*sin
                # (partition-shifted reads are legal when one input is PSUM;
                # sin_t carries the -sin/+sin half signs)
                t1 = rope_pool.tile([P, 512], f16, tag="t1")
                t2 = rope_pool.tile([P, 512], f16, tag="t2")
                nc.vector.tensor_mul(t1[:], ps[:], cos_sb[:, t0 : t0 + 512])
                nc.vector.tensor_mul(
                    t2[0:64, :], ps[64:128, :], sin_sb[0:64, t0 : t0 + 512]
                )
                nc.vector.tensor_mul(
                    t2[64:128, :], ps[0:64, :], sin_sb[64:128, t0 : t0 + 512]
                )
                nc.vector.tensor_add(dstT[:, hh, t0 : t0 + 512], t1[:], t2[:])
            # v: natural [tok, D*2] orientation (lhsT = xT tile)
            for sub in range(4):
                vps = work_pool.tile([P, 2 * P], f32, tag="work", name="vps")
                for ct in range(CT):
                    nc.tensor.matmul(
                        vps[:],
                        xt[:, ct, sub * P : (sub + 1) * P],
                        wv_sb[:, ct, :],
                        start=(ct == 0),
                        stop=(ct == CT - 1),
                    )
                tt = ti * 4 + sub
                # both heads in one strided copy, on the (phase-B-idle)
                # scalar engine
                nc.scalar.copy(v_sb[:, tt, :, 0:D], vps[:])

        # ======== phase C: causal attention per (b, h, qi) unit ========
        # S^T per k-tile: [k128, q512]. Diagonal k-tiles restricted to the
        # valid causal column range [g, 512). exp'd probabilities (ptd) are
        # buffered in SBUF for the whole q-row, so PV runs as one PSUM
        # accumulation chain per 128-token sub -- two 1-bank O accumulators
        # suffice. O stays UNNORMALIZED: only the rowsum reciprocal is saved
        # (per token) and folded into the phase-D drain.
        #
        # Loop nesting b -> qi -> h so both heads of a token tile finish
        # together (phase D's tt work becomes ready early and evenly).
        # O-drain emission is deferred by one (qi, h) step so the serial
        # psum->sbuf->transpose->sbuf hops never head-of-line-block the PE.
        pending = []

        def emit_pending():
            for o_sb, tt, h in pending:
                tp = cs_pool.tile([P, P], f16, tag="cs", name="tp")
                nc.tensor.transpose(tp[:], o_sb[:], id_sb[:])
                nc.vector.tensor_copy(oT_sb[:, tt, h, :], tp[:])
            pending.clear()

        for b in range(B):
            for qi in range(NQ):
                for h in range(HPC):
                    toff = b * T
                    q0 = toff + qi * 512
                    ndiag0 = qi * 4  # first diagonal kt
                    nkt = ndiag0 + 4
                    ptds = []
                    for kt in range(nkt):
                        k0 = toff + kt * P
                        gi = kt - ndiag0
                        g = max(gi, 0) * P
                        w = 512 - g
                        sd = cs_pool.tile([P, 512], f32, tag="cs", name="sd")
                        nc.tensor.matmul(
                            sd[:, 0:w],
                            kT_sb[:, h, k0 : k0 + P],
                            qT_sb[:, h, q0 + g : q0 + 512],
                            start=True,
                            stop=True,
                        )
                        ptd = p_pool.tile([P, 512], f16, tag="pt", name="ptd")
                        nc.scalar.activation(
                            ptd[:, 0:w], sd[:, 0:w], Exp, scale=float(SCALE)
                        )
                        if gi >= 0:  # diagonal: multiplicative causal mask
                            nc.vector.tensor_mul(
                                ptd[:, 0:w], ptd[:, 0:w], mask_sb[:, 384 : 384 + w]
                            )
                        ptds.append((ptd, g))
                    emit_pending()
                    # PV: per-sub accumulation chains over the buffered ptds
                    for s in range(4):
                        last = ndiag0 + s
                        ot = co_pool.tile([P, D + 1], f32, tag="co", name="ot")
                        for kt in range(last + 1):
                            ptd, g = ptds[kt]
                            nc.tensor.matmul(
                                ot[:],
                                ptd[:, s * P - g : s * P - g + P],
                                v_sb[:, b * 16 + kt, h, :],
                                start=(kt == 0),
                                stop=(kt == last),
                            )
                        # free the co slot promptly: recip + normalizing
                        # psum->sbuf drain now (the per-head 1/rowsum is
                        # fused into the copy -- same DVE cost as a plain
                        # copy); transpose + oT copy deferred one step
                        tt = b * 16 + qi * 4 + s
                        rtmp = osb_pool.tile([P, 1], f32, tag="rtmp")
                        nc.vector.reciprocal(rtmp[:], ot[:, D : D + 1])
                        o_sb = osb_pool.tile([P, P], f16, tag="osb")
                        nc.vector.tensor_scalar_mul(o_sb[:], ot[:, 0:D], rtmp[:])
                        pending.append((o_sb, tt, h))
        emit_pending()

        # deferred wproj load (only needed for phase D)
        nc.sync.dma_start(wproj_sb[:], wproj.rearrange("(h p) m -> p h m", p=P))

        # ======== phase D: output projection ========
        # 2-bank yps tiles: 4 chained matmuls per drain (two column-pair
        # accumulation groups, each within its own bank), then one wide
        # cast drain (scalar or DVE) and one 256KB DMA.
        for tt in range(TT):
            for m in range(2):
                yps = work_pool.tile([P, 1024], f32, tag="work", name="yps")
                for h2 in range(2):
                    cc = m * 2 + h2
                    for h in range(HPC):
                        nc.tensor.matmul(
                            yps[:, h2 * 512 : (h2 + 1) * 512],
                            oT_sb[:, tt, h, :],
                            wproj_sb[:, h, cc * 512 : (cc + 1) * 512],
                            start=(h == 0),
                            stop=(h == HPC - 1),
                        )
                ysb = ysb_pool.tile([P, 1024], f16, tag="ysb")
                if (tt * 2 + m) % 4 == 0:
                    nc.scalar.copy(ysb[:], yps[:])
                else:
                    nc.vector.tensor_copy(ysb[:], yps[:])
                nc.sync.dma_start(
                    y[tt * P : (tt + 1) * P, m * 1024 : (m + 1) * 1024], ysb[:]
                )

    # bacc lowering: splits multi-sem waits into EventSemaphore insts
    # (TRN2 allows at most 1 wait per regular instruction), reg alloc, DCE.
    nc.compile()
    return nc


def _host_inputs(x, Wqkv, Wproj):
    """Build per-core device input maps (host-side sharding)."""
    xTf = np.ascontiguousarray(x.reshape(NTOK, C).T).astype(np.float16)

    invf = 1.0 / (10000.0 ** (np.arange(0, D, 2, dtype=np.float32) / D))
    freqs = np.arange(T, dtype=np.float32)[:, None] * invf[None, :]  # [T, 64]
    cos = np.cos(freqs).astype(np.float32).T  # [64, T]
    sin = np.sin(freqs).astype(np.float32).T
    cos_t = np.tile(np.concatenate([cos, cos], axis=0), (1, B)).astype(np.float16)
    sin_t = np.tile(np.concatenate([-sin, sin], axis=0), (1, B)).astype(np.float16)

    ii = np.arange(P)[:, None]
    mm = np.arange(1024)[None, :]
    maskw = (mm >= ii + 384).astype(np.float16)
    ident = np.eye(P, dtype=np.float16)

    in_maps = []
    for c in range(NCORES):
        h0 = c * HPC * D  # col offset of this core's heads
        wqk_c = np.concatenate(
            [Wqkv[:, h0 : h0 + HPC * D], Wqkv[:, C + h0 : C + h0 + HPC * D]], axis=1
        ).astype(np.float16)
        wv_c = Wqkv[:, 2 * C + h0 : 2 * C + h0 + HPC * D].astype(np.float16)
        wproj_c = np.ascontiguousarray(Wproj[h0 : h0 + HPC * D, :]).astype(np.float16)
        in_maps.append(
            {
                "xT": xTf,
                "wqk": np.ascontiguousarray(wqk_c),
                "wv": np.ascontiguousarray(wv_c),
                "wproj": wproj_c,
                "cos_t": cos_t,
                "sin_t": sin_t,
                "maskw": maskw,
                "ident": ident,
            }
        )
    return in_maps


def kernel(x, Wqkv, Wproj, _trace=False):
    global _compiled
    x = np.asarray(x, dtype=np.float32)
    Wqkv = np.asarray(Wqkv, dtype=np.float32)
    Wproj = np.asarray(Wproj, dtype=np.float32)

    from concourse.bass_utils import run_bass_kernel_spmd

    if _compiled is None:
        _compiled = _build_bass()
    nc = _compiled

    in_maps = _host_inputs(x, Wqkv, Wproj)
    res = run_bass_kernel_spmd(nc, in_maps, list(range(NCORES)), trace=_trace)
    out = np.zeros((NTOK, C), dtype=np.float32)
    for r in res.results:
        out += r["y"].astype(np.float32)
    kernel._last_result = res
    return out.reshape(B, T, C)



# revision 21
# speedup vs baseline: 1.1470x; 1.1470x over previous
"""Causal self-attention (B=2, T=2048, C=2048, H=16, rope) on 8 trn2 cores.

Sharding: tensor-parallel over heads. Each core owns 2 of 16 heads:
  - Wqkv columns for its heads (q,k,v), Wproj rows for its heads.
  - Computes qkv projection, rope, causal attention, and its partial
    output projection y_c = O_c @ Wproj_c  (full [4096, 2048]).
  - Host sums the 8 partials (the all-reduce / unshard for row-parallel TP).

v2: single software-pipelined loop. Step s emits C(s-1) attention (exp-gated,
low PE density), then B(s) qkv+rope (dense PE filler), then D(s-2) output
projection (more filler). The tile list-scheduler weaves the streams so the
PE never starves while the scalar engine chews exp. Further: PE prewarm
matmuls cover the initial x/w DMA window (the cost model halves matmul rate
for ~3us after any PE idle), all weights are host-pretransposed into exact
SBUF layouts so every DMA is wide-line contiguous, S tiles are paired into
2-bank PSUM tiles so exp runs 1024-wide (halves ACT op overhead), and y
output DMAs ride the second (Activation) HWDGE queue so stores never delay
input streams.
"""

import sys

for _p in ("/opt/trn_rl_repo",):
    if _p not in sys.path:
        sys.path.append(_p)

import numpy as np

# ---- problem constants (hardcoded per the task contract) ----
B, T, C, H = 2, 2048, 2048, 16
D = C // H  # 128
NCORES = 8
HPC = H // NCORES  # heads per core = 2
NTOK = B * T  # 4096
P = 128
CT = C // P  # 16 contraction tiles
TOK512 = NTOK // 512  # 8
NQ = T // 512  # q-tiles per unit = 4
TT = NTOK // P  # 32 token 128-tiles
SCALE = 1.0 / np.sqrt(D)

_compiled = None

# tuning knobs
KNOBS = {
    "warm": 32,      # prewarm matmul count (N=256 each)
    "xt": 2,         # xt prefetch depth
    "ptp": 14,       # live exp'd S pair-tiles
    "osb": 8,
    "ysb": 4,        # [128, 4*512] whole-tt staging tiles
    "rope": 2,
    "ydma_q": "act", # which HWDGE queue carries y stores
    "workb": 4,
    "sdb": 1,
    "drain": "vavv",
    "maskeng": "v",
}


def _build_bass():
    import concourse.bacc as bacc
    import concourse.mybir as mybir
    import concourse.tile as tile
    from contextlib import ExitStack

    f16 = mybir.dt.float16
    f32 = mybir.dt.float32
    Exp = mybir.ActivationFunctionType.Exp

    nc = bacc.Bacc()

    xp = nc.declare_dram_parameter("xp", [P, TOK512 * CT * 512], f16, isOutput=False)
    wqk = nc.declare_dram_parameter("wqk", [P, 4 * CT * P], f16, isOutput=False)
    wv = nc.declare_dram_parameter("wv", [P, CT * HPC * D], f16, isOutput=False)
    wproj = nc.declare_dram_parameter("wproj", [P, HPC * C], f16, isOutput=False)
    cos_t = nc.declare_dram_parameter("cos_t", [P, NTOK], f16, isOutput=False)
    sin_t = nc.declare_dram_parameter("sin_t", [P, NTOK], f16, isOutput=False)
    maskw = nc.declare_dram_parameter("maskw", [P, 1024], f16, isOutput=False)
    ident = nc.declare_dram_parameter("ident", [P, P], f16, isOutput=False)
    y = nc.declare_dram_parameter("y", [NTOK, C], f16, isOutput=True)

    with tile.TileContext(nc) as tc, ExitStack() as ctx:
        pers = ctx.enter_context(tc.tile_pool(name="pers", bufs=1))

        # ---- persistent SBUF tensors ----
        wqk_sb = pers.tile([P, 4, CT, P], f16)  # [c128, ci(q0,q1,k0,k1), ct, m]
        wv_sb = pers.tile([P, CT, HPC * D], f16)
        wproj_sb = pers.tile([P, HPC, C], f16)
        cos_sb = pers.tile([P, NTOK], f16)
        sin_sb = pers.tile([P, NTOK], f16)
        mask_sb = pers.tile([P, 1024], f16)
        id_sb = pers.tile([P, P], f16)
        scratch = pers.tile([P, 256], f16)  # prewarm operand
        qT_sb = pers.tile([P, HPC, NTOK], f16)  # [d, h, tok] rope'd
        kT_sb = pers.tile([P, HPC, NTOK], f16)
        v_sb = pers.tile([P, TT, HPC, D + 1], f16)  # [tokmod, tt, h, D|ones]
        oT_sb = pers.tile([P, TT, HPC, P], f16)  # [d, tt, h, tokmod]

        xt_pool = ctx.enter_context(tc.tile_pool(name="xt", bufs=KNOBS["xt"]))
        rope_pool = ctx.enter_context(tc.tile_pool(name="rope", bufs=KNOBS["rope"]))
        ptp_pool = ctx.enter_context(tc.tile_pool(name="ptp", bufs=KNOBS["ptp"]))
        osb_pool = ctx.enter_context(tc.tile_pool(name="osb", bufs=KNOBS["osb"]))
        ysb_pool = ctx.enter_context(tc.tile_pool(name="ysb", bufs=KNOBS["ysb"]))
        # PSUM budget (8 banks): work 2x1 + sd 2x2 + co 2x1 = 8
        work_pool = ctx.enter_context(tc.tile_pool(name="work", bufs=KNOBS["workb"], space="PSUM"))
        sd_pool = ctx.enter_context(tc.tile_pool(name="sd", bufs=KNOBS["sdb"], space="PSUM"))
        co_pool = ctx.enter_context(tc.tile_pool(name="co", bufs=2, space="PSUM"))

        ydma = nc.scalar if KNOBS["ydma_q"] == "act" else nc.sync

        # ---- upfront DMA stream (SP HWDGE queue), first-use order ----
        xts = [None] * TOK512

        def load_xt(ti):
            xt = xt_pool.tile([P, CT, 512], f16, tag="xt", name="xt")
            base = ti * CT * 512
            nch = 4 if ti == 0 else 2
            step = CT // nch
            for c in range(nch):
                nc.sync.dma_start(
                    xt[:, c * step : (c + 1) * step, :],
                    xp[:, base + c * step * 512 : base + (c + 1) * step * 512],
                )
            if ti > 0:
                t0 = ti * 512
                nc.sync.dma_start(cos_sb[:, t0 : t0 + 512], cos_t[:, t0 : t0 + 512])
                nc.sync.dma_start(sin_sb[:, t0 : t0 + 512], sin_t[:, t0 : t0 + 512])
            xts[ti] = xt

        nc.sync.dma_start(id_sb[:], ident[:])
        nc.sync.dma_start(wqk_sb[:, 0, :, :], wqk[:, 0 : CT * P])
        load_xt(0)
        nc.sync.dma_start(wqk_sb[:, 1, :, :], wqk[:, CT * P : 2 * CT * P])
        nc.sync.dma_start(cos_sb[:, 0:512], cos_t[:, 0:512])
        nc.sync.dma_start(sin_sb[:, 0:512], sin_t[:, 0:512])
        for ci in range(2, 4):
            nc.sync.dma_start(
                wqk_sb[:, ci, :, :], wqk[:, ci * CT * P : (ci + 1) * CT * P]
            )
        nc.sync.dma_start(mask_sb[:], maskw[:])
        nc.sync.dma_start(wv_sb[:], wv[:])
        load_xt(1)
        nc.sync.dma_start(wproj_sb[:], wproj[:])

        # ---- PE prewarm: keep the ramp model warm through the cold DMA window
        nc.vector.memset(scratch[:], 0.0)
        for _ in range(KNOBS["warm"]):
            wps = work_pool.tile([P, 256], f32, tag="work", name="warm")
            nc.tensor.matmul(wps[:], scratch[:, 0:P], scratch[:], start=True, stop=True)

        nc.vector.memset(v_sb[:, :, :, D : D + 1], 1.0)

        # ---- helpers ----
        def emit_B(ti):
            t0 = ti * 512
            if xts[ti] is None:
                load_xt(ti)
            if ti + 2 < TOK512 and xts[ti + 2] is None:
                load_xt(ti + 2)
            xt = xts[ti]
            for ci in range(4):
                hh = ci % HPC
                dstT = qT_sb if ci < HPC else kT_sb
                ps = work_pool.tile([P, 512], f32, tag="work", name="psqk")
                for ct in range(CT):
                    nc.tensor.matmul(
                        ps[:],
                        wqk_sb[:, ci, ct, :],
                        xt[:, ct, :],
                        start=(ct == 0),
                        stop=(ct == CT - 1),
                    )
                # rope straight off PSUM: t1 = ps*cos; t2 = roll64(ps)*sin
                t1 = rope_pool.tile([P, 512], f16, tag="t1")
                t2 = rope_pool.tile([P, 512], f16, tag="t2")
                nc.vector.tensor_mul(t1[:], ps[:], cos_sb[:, t0 : t0 + 512])
                nc.vector.tensor_mul(
                    t2[0:64, :], ps[64:128, :], sin_sb[0:64, t0 : t0 + 512]
                )
                nc.vector.tensor_mul(
                    t2[64:128, :], ps[0:64, :], sin_sb[64:128, t0 : t0 + 512]
                )
                nc.vector.tensor_add(dstT[:, hh, t0 : t0 + 512], t1[:], t2[:])
            for sub in range(4):
                vps = work_pool.tile([P, HPC * D], f32, tag="work", name="vps")
                for ct in range(CT):
                    nc.tensor.matmul(
                        vps[:],
                        xt[:, ct, sub * P : (sub + 1) * P],
                        wv_sb[:, ct, :],
                        start=(ct == 0),
                        stop=(ct == CT - 1),
                    )
                tt = ti * 4 + sub
                nc.scalar.copy(v_sb[:, tt, :, 0:D], vps[:])

        pending = []

        def emit_pending():
            for o_sb, tt, h in pending:
                tp = co_pool.tile([P, P], f16, tag="co", name="tp")
                nc.tensor.transpose(tp[:], o_sb[:], id_sb[:])
                nc.vector.tensor_copy(oT_sb[:, tt, h, :], tp[:])
            pending.clear()

        def emit_C(u):
            b, qi = u // NQ, u % NQ
            toff = b * T
            q0 = toff + qi * 512
            ndiag0 = qi * 4
            nkt = ndiag0 + 4
            emit_pending()
            ptds_h = []
            for h in range(HPC):
                ptds = []  # (ptp tile, flat base col, g) per kt
                # full tiles, exp'd in 1024-wide pairs
                for pk in range(ndiag0 // 2):
                    sd = sd_pool.tile([P, 1024], f32, tag="sd", name="sd")
                    ptp = ptp_pool.tile([P, 1024], f16, tag="ptp", name="ptp")
                    for j in range(2):
                        kt = pk * 2 + j
                        k0 = toff + kt * P
                        nc.tensor.matmul(
                            sd[:, j * 512 : (j + 1) * 512],
                            kT_sb[:, h, k0 : k0 + P],
                            qT_sb[:, h, q0 : q0 + 512],
                            start=True,
                            stop=True,
                        )
                        ptds.append((ptp, j * 512, 0))
                    nc.scalar.activation(ptp[:], sd[:], Exp, scale=float(SCALE))
                # diagonal tiles packed: (w512@0, w384@512) and (w256@0, w128@256)
                for pk, packs in ((0, ((0, 0), (1, 512))), (1, ((2, 0), (3, 256)))):
                    sd = sd_pool.tile([P, 1024], f32, tag="sd", name="sd")
                    ptp = ptp_pool.tile([P, 1024], f16, tag="ptp", name="ptp")
                    wtot = 0
                    for gi, base in packs:
                        kt = ndiag0 + gi
                        k0 = toff + kt * P
                        g = gi * P
                        w = 512 - g
                        nc.tensor.matmul(
                            sd[:, base : base + w],
                            kT_sb[:, h, k0 : k0 + P],
                            qT_sb[:, h, q0 + g : q0 + 512],
                            start=True,
                            stop=True,
                        )
                        ptds.append((ptp, base, g))
                        wtot = base + w
                    nc.scalar.activation(
                        ptp[:, 0:wtot], sd[:, 0:wtot], Exp, scale=float(SCALE)
                    )
                    for gi, base in packs:
                        w = 512 - gi * P
                        nc.vector.tensor_mul(
                            ptp[:, base : base + w],
                            ptp[:, base : base + w],
                            mask_sb[:, 384 : 384 + w],
                        )
                ptds_h.append(ptds)
            for h in range(HPC):
                ptds = ptds_h[h]
                for s in range(4):
                    last = ndiag0 + s
                    ot = co_pool.tile([P, D + 1], f32, tag="co", name="ot")
                    for kt in range(last + 1):
                        ptp, base, g = ptds[kt]
                        c0 = base + s * P - g
                        nc.tensor.matmul(
                            ot[:],
                            ptp[:, c0 : c0 + P],
                            v_sb[:, b * 16 + kt, h, :],
                            start=(kt == 0),
                            stop=(kt == last),
                        )
                    tt = b * 16 + qi * 4 + s
                    rtmp = osb_pool.tile([P, 1], f32, tag="rtmp")
                    nc.vector.reciprocal(rtmp[:], ot[:, D : D + 1])
                    o_sb = osb_pool.tile([P, P], f16, tag="osb")
                    nc.vector.tensor_scalar_mul(o_sb[:], ot[:, 0:D], rtmp[:])
                    pending.append((o_sb, tt, h))

        def emit_D(td, tt4s):
            for tt4 in tt4s:
                tt = td * 4 + tt4
                ysb = ysb_pool.tile([P, 4, 512], f16, tag="ysb")
                for cc in range(4):
                    yps = work_pool.tile([P, 512], f32, tag="work", name="yps")
                    for h in range(HPC):
                        nc.tensor.matmul(
                            yps[:],
                            oT_sb[:, tt, h, :],
                            wproj_sb[:, h, cc * 512 : (cc + 1) * 512],
                            start=(h == 0),
                            stop=(h == HPC - 1),
                        )
                    if KNOBS["drain"][cc] == "a":
                        nc.scalar.copy(ysb[:, cc, :], yps[:])
                    else:
                        nc.vector.tensor_copy(ysb[:, cc, :], yps[:])
                ydma.dma_start(y[tt * P : (tt + 1) * P, :], ysb[:])

        # ---- the pipeline ----
        for step in range(TOK512 + 3):
            if 1 <= step <= TOK512:
                emit_C(step - 1)
            if step < TOK512:
                emit_B(step)
            if 2 <= step <= TOK512 + 1:
                emit_D(step - 2, (0, 1))
            if step >= 3:
                emit_D(step - 3, (2, 3))
        emit_pending()

    nc.compile()
    return nc


def _host_inputs(x, Wqkv, Wproj):
    """Build per-core device input maps (host-side sharding + pre-layout)."""
    xTf = np.ascontiguousarray(x.reshape(NTOK, C).T).astype(np.float16)
    # [p][ti][ct][j] : xp[p, ti, ct, j] = xT[ct*128+p, ti*512+j]
    xp = np.ascontiguousarray(
        xTf.reshape(CT, P, TOK512, 512).transpose(1, 2, 0, 3).reshape(P, -1)
    )

    invf = 1.0 / (10000.0 ** (np.arange(0, D, 2, dtype=np.float32) / D))
    freqs = np.arange(T, dtype=np.float32)[:, None] * invf[None, :]  # [T, 64]
    cos = np.cos(freqs).astype(np.float32).T  # [64, T]
    sin = np.sin(freqs).astype(np.float32).T
    cos_t = np.tile(np.concatenate([cos, cos], axis=0), (1, B)).astype(np.float16)
    sin_t = np.tile(np.concatenate([-sin, sin], axis=0), (1, B)).astype(np.float16)

    ii = np.arange(P)[:, None]
    mm = np.arange(1024)[None, :]
    maskw = (mm >= ii + 384).astype(np.float16)
    ident = np.eye(P, dtype=np.float16)

    in_maps = []
    for c in range(NCORES):
        h0 = c * HPC * D  # col offset of this core's heads
        wqk_c = np.concatenate(
            [Wqkv[:, h0 : h0 + HPC * D], Wqkv[:, C + h0 : C + h0 + HPC * D]], axis=1
        ).astype(np.float16)  # [2048, 512] cols = q0,q1,k0,k1
        # [p][ci][ct][m]
        wqk_p = np.ascontiguousarray(
            wqk_c.reshape(CT, P, 4, P).transpose(1, 2, 0, 3).reshape(P, -1)
        )
        wv_c = Wqkv[:, 2 * C + h0 : 2 * C + h0 + HPC * D].astype(np.float16)
        wv_p = np.ascontiguousarray(
            wv_c.reshape(CT, P, HPC * D).transpose(1, 0, 2).reshape(P, -1)
        )
        wproj_c = Wproj[h0 : h0 + HPC * D, :].astype(np.float16)
        wproj_p = np.ascontiguousarray(
            wproj_c.reshape(HPC, P, C).transpose(1, 0, 2).reshape(P, -1)
        )
        in_maps.append(
            {
                "xp": xp,
                "wqk": wqk_p,
                "wv": wv_p,
                "wproj": wproj_p,
                "cos_t": cos_t,
                "sin_t": sin_t,
                "maskw": maskw,
                "ident": ident,
            }
        )
    return in_maps


def kernel(x, Wqkv, Wproj, _trace=False):
    global _compiled
    x = np.asarray(x, dtype=np.float32)
    Wqkv = np.asarray(Wqkv, dtype=np.float32)
    Wproj = np.asarray(Wproj, dtype=np.float32)

    from concourse.bass_utils import run_bass_kernel_spmd

    if _compiled is None:
        _compiled = _build_bass()
    nc = _compiled

    in_maps = _host_inputs(x, Wqkv, Wproj)
    res = run_bass_kernel_spmd(nc, in_maps, list(range(NCORES)), trace=_trace)
    out = np.zeros((NTOK, C), dtype=np.float32)
    for r in res.results:
        out += r["y"].astype(np.float32)
    kernel._last_result = res
    return out.reshape(B, T, C)
